# revision 38
# baseline (speedup 1.0000x reference)
"""Trainium2 Bass kernel for nn_Attention_25383256719981.

Dense transformer attention block:
  qkv = x @ W_qkv.T ; rotary(q,k,v) ; causal+padding-masked softmax(q k^T / sqrt(dh)) @ v ;
  out = heads @ W_out.T + b_out

Sharding: tensor-parallel over heads. 16 heads / 8 cores = 2 heads per core.
Each core computes its 2 heads' QKV projection, attention, and a partial
output projection (y_partial = O_heads @ W_out[:, head_cols].T); the host
sums the 8 f16 partials in f32 and adds b_out.

Key device-side design (v2, tuned from the NTFF profile):
  - dh dims stored DEINTERLEAVED ([evens|odds] per head, via host-side
    permutation of W_qkv rows / W_out cols / rotary tables), so the rotary
    pair-shuffle becomes dense half-block multiplies: 4 packed-f16 SBUF DVE
    ops per token block (eligible for the DVE 2x/4x perf modes).
  - cos/sin tables precomputed on host (f16), sin sign-folded in partner
    layout: rot(x) = x*cosd + swap_halves(x)*sind.
  - QKV PSUM tiles evacuated to SBUF f16 by ScalarE (batch 0) / Pool
    (batch 1, woven into attention while ScalarE runs exp).
  - Single QT/KT [128, N] per batch (head h occupies dh rows h*64:h*64+64);
    S matmuls run K=64 at base partition h*64 (same per-row cost as K=128).
  - S for both heads lands in one [128, 1024] 2-bank PSUM tile; ONE exp
    activation [128, 2, w] covers both heads; probabilities pt in f16 SBUF.
  - O^T accumulated per (head, qc) with an appended ones-row ([V|1]^T P^T)
    giving softmax row sums Z; 1/Z via DVE reciprocal straight on the PSUM
    row + gpsimd partition_broadcast (no DMA round trip).
  - Emission order software-pipelines the whole kernel to keep the PE
    continuously busy (DVFS ramp): [b0 QKV+transposes] [b0 attention with
    b1 QKV woven in] [b1 attention + projections].
  - y written as f16 (halves the partial-sum HBM traffic).
"""

import sys

import numpy as np

for _p in ("/opt/trn_rl_repo",):
    if _p not in sys.path:
        sys.path.insert(0, _p)

import concourse.bass as bass
import concourse.bacc as bacc
import concourse.mybir as mybir
import concourse.tile as tile
from concourse.bass_utils import run_bass_kernel_spmd
from concourse.masks import make_identity

# Problem shapes (hardcoded per contract).
B, N, D, H, DH = 2, 2048, 1024, 16, 64
NCORES = 8
HPC = H // NCORES            # heads per core
P = 128
NT = B * N                   # total tokens
SCALE = DH ** -0.5
FD = HPC * DH                # per-core features per tensor (128)
F3 = 3 * FD                  # 384
NEG = -1.0e30
NB = N // P                  # 16 token-blocks per batch
NCH = N // 512               # 4 x-chunks of 512 tokens per batch
KO = D // P                  # 8 contraction blocks

f32 = mybir.dt.float32
f16 = mybir.dt.float16
AF = mybir.ActivationFunctionType
ALU = mybir.AluOpType


def build_nc():
    nc = bacc.Bacc("TRN2", target_bir_lowering=False)

    xT = nc.dram_tensor("xT", [D, NT], f16, kind="ExternalInput")
    wqkvT = nc.dram_tensor("wqkvT", [D, F3], f16, kind="ExternalInput")
    woT = nc.dram_tensor("woT", [FD, D], f16, kind="ExternalInput")
    cosd = nc.dram_tensor("cosd", [N, DH], f16, kind="ExternalInput")
    sind = nc.dram_tensor("sind", [N, DH], f16, kind="ExternalInput")
    madd = nc.dram_tensor("madd", [P, B * NB], f32, kind="ExternalInput")
    caus = nc.dram_tensor("caus", [P, P], f16, kind="ExternalInput")
    y = nc.dram_tensor("y", [NT, D], f16, kind="ExternalOutput")

    xT_r = xT.rearrange("(ko p) t -> p ko t", p=P)          # [128, 8, 4096]
    wq_r = wqkvT.rearrange("(ko p) f -> p ko f", p=P)       # [128, 8, 384]
    cos_r = cosd.rearrange("(t p) d -> p t d", p=P)         # [128, 16, 64]
    sin_r = sind.rearrange("(t p) d -> p t d", p=P)

    with tile.TileContext(nc) as tc, \
            tc.tile_pool(name="const", bufs=1) as const, \
            tc.tile_pool(name="xp", bufs=2 * NCH) as xp, \
            tc.tile_pool(name="qsb", bufs=2) as qsb, \
            tc.tile_pool(name="tmpp", bufs=2) as tmpp, \
            tc.tile_pool(name="qkbp", bufs=2) as qkbp, \
            tc.tile_pool(name="vfp", bufs=2) as vfp, \
            tc.tile_pool(name="qtp", bufs=2) as qtp, \
            tc.tile_pool(name="ptp", bufs=4) as ptp, \
            tc.tile_pool(name="plp", bufs=2) as plp, \
            tc.tile_pool(name="zp", bufs=4) as zp, \
            tc.tile_pool(name="ysb", bufs=4) as ysb, \
            tc.tile_pool(name="psS", bufs=2, space="PSUM") as psS, \
            tc.tile_pool(name="psM", bufs=2, space="PSUM") as psM, \
            tc.tile_pool(name="psO", bufs=2, space="PSUM") as psO:

        # ---- constants / weights (x chunks first: QKV needs them first) ---
        w_sb = const.tile([P, KO, F3], f16, tag="w")
        nc.sync.dma_start(w_sb[:, :, :], wq_r)
        x_sbs = {}
        for b in range(B):
            for c in range(NCH):
                x_sb = xp.tile([P, KO, 512], f16, tag="x", name=f"x_{b}_{c}")
                tok0 = b * N + c * 512
                nc.sync.dma_start(x_sb[:, :, :], xT_r[:, :, tok0:tok0 + 512])
                x_sbs[(b, c)] = x_sb
        wo_sb = const.tile([FD, D], f16, tag="wo")
        nc.sync.dma_start(wo_sb[:, :], woT[:, :])
        caus01 = const.tile([P, P], f16, tag="caus01")
        nc.sync.dma_start(caus01[:, :], caus[:, :])
        madd_sb = const.tile([P, B * NB], f32, tag="madd")
        nc.sync.dma_start(madd_sb[:, :], madd[:, :])
        cos_sb = const.tile([P, NB, DH], f16, tag="cos")
        nc.sync.dma_start(cos_sb[:, :, :], cos_r)
        sin_sb = const.tile([P, NB, DH], f16, tag="sin")
        nc.sync.dma_start(sin_sb[:, :, :], sin_r)
        ident = const.tile([P, P], f16, tag="ident")
        make_identity(nc, ident)
        onecol = const.tile([P, 1], f32, tag="onecol")
        nc.gpsimd.memset(onecol, 1.0)

        # Per-batch persistent tiles.
        QT = {}
        KT = {}
        VF = {}
        for b in range(B):
            QT[b] = qtp.tile([P, N], f16, tag="QT", name=f"QT{b}")
            KT[b] = qtp.tile([P, N], f16, tag="KT", name=f"KT{b}")
            VF[b] = vfp.tile([P, NB, HPC * (DH + 1)], f16, tag="vf",
                             name=f"VF{b}")

        def emit_vf_ones(b):
            nc.vector.tensor_copy(
                VF[b][:, :, DH::DH + 1],
                onecol[:, None, :].to_broadcast([P, NB, HPC]))

        def ecopy(eng, out, in_):
            if eng is nc.scalar:
                eng.copy(out, in_)
            else:
                eng.tensor_copy(out, in_)

        # ---------------- QKV + rotary, one 512-token chunk ---------------
        # QKV accumulators live in the (phase-1-idle) "s" slots.  The rotary
        # elementwise work is batched per CHUNK (4 blocks per instruction):
        # Pool's ~0.7us/instr semaphore overhead made per-block ops too slow
        # to keep up with the PE.  Evacs on ScalarE; multiplies DVE; adds
        # Pool; out = x*cos + swap_halves(x)*sind.
        def emit_qkv_block_mm(b, c, tb, qsb_c, ps_tag, evac_eng):
            x_sb = x_sbs[(b, c)]
            if ps_tag == "s":
                qkv_ps = psS.tile([P, 1024], f32, tag="s",
                                  name=f"qkv_{b}_{c}_{tb}")
            else:
                qkv_ps = psM.tile([P, 512], f32, tag="mm",
                                  name=f"qkv_{b}_{c}_{tb}")
            for ko in range(KO):
                nc.tensor.matmul(
                    qkv_ps[:, 0:F3],
                    x_sb[:, ko, tb * P:(tb + 1) * P],
                    w_sb[:, ko, :],
                    start=(ko == 0), stop=(ko == KO - 1),
                )
            ecopy(evac_eng, qsb_c[:, tb, :], qkv_ps[:, 0:F3])

        def emit_qkv_chunk_mm(b, c):
            qsb_c = qsb.tile([P, 4, F3], f16, tag="qs", name=f"qsb_{b}_{c}")
            for tb in range(4):
                emit_qkv_block_mm(b, c, tb, qsb_c, "s", nc.scalar)
            return qsb_c

        def emit_rotary_chunk(b, c, qsb_c, cos_eng=None):
            t0 = c * 4
            g6 = qsb_c.rearrange("p f (g d) -> p f g d", g=6)
            tmp = tmpp.tile([P, 4, F3], f16, tag="tmp", name=f"tmp_{b}_{c}")
            t6 = tmp.rearrange("p f (g d) -> p f g d", g=6)
            se = sin_sb[:, t0:t0 + 4, None, 0:32].to_broadcast([P, 4, 6, 32])
            so = sin_sb[:, t0:t0 + 4, None, 32:64].to_broadcast([P, 4, 6, 32])
            nc.vector.tensor_tensor(t6[:, :, :, 0:32], g6[:, :, :, 32:64],
                                    se, ALU.mult)
            nc.vector.tensor_tensor(t6[:, :, :, 32:64], g6[:, :, :, 0:32],
                                    so, ALU.mult)
            cq = tmpp.tile([P, 4, F3], f16, tag="cq", name=f"cq_{b}_{c}")
            c6 = cq.rearrange("p f (g d) -> p f g d", g=6)
            cb = cos_sb[:, t0:t0 + 4, None, :].to_broadcast([P, 4, 6, DH])
            (cos_eng or nc.vector).tensor_tensor(c6, g6, cb, ALU.mult)
            qkb = qkbp.tile([P, 4, 2 * FD], f16, tag="qkb", name=f"qkb_{b}_{c}")
            nc.gpsimd.tensor_tensor(qkb, tmp[:, :, 0:2 * FD],
                                    cq[:, :, 0:2 * FD], ALU.add)
            vf_v = VF[b][:, t0:t0 + 4, :].rearrange(
                "p f (h c) -> p f h c", h=HPC)[:, :, :, 0:DH]
            tmp_v = tmp[:, :, 2 * FD:F3].rearrange("p f (h d) -> p f h d", h=HPC)
            cq_v = cq[:, :, 2 * FD:F3].rearrange("p f (h d) -> p f h d", h=HPC)
            nc.gpsimd.tensor_tensor(vf_v, tmp_v, cq_v, ALU.add)
            return qkb

        # transposes q-pair and k-pair -> QT/KT columns for one block
        def emit_tr_block(b, c, tb, qkb, copy_engs):
            t = c * 4 + tb
            for which, dst in ((0, QT[b]), (1, KT[b])):
                tr_ps = psM.tile([P, P], f16, tag="mm",
                                 name=f"tr{which}_{b}_{t}")
                nc.tensor.transpose(
                    tr_ps, qkb[:, tb, which * P:(which + 1) * P], ident)
                ecopy(copy_engs[which % len(copy_engs)],
                      dst[:, t * P:(t + 1) * P], tr_ps)

        def emit_tr_chunk(b, c, qkb, copy_engs):
            for tb in range(4):
                emit_tr_block(b, c, tb, qkb, copy_engs)

        # ---------------- attention -----------------------------------
        # Per batch: 40 (qc, kb) pairs; S pair -> exp -> (lag 2) O pair.
        def attn_pairs(b):
            return [(qc, kb) for qc in range(NCH) for kb in range(4 * qc + 4)]

        def emit_S(b, qc, kb):
            qs = max(kb * P, 512 * qc)
            off = qs - 512 * qc
            w = 512 - off
            S_t = psS.tile([P, 1024], f32, tag="s", name=f"S_{b}_{qc}_{kb}")
            for h in range(HPC):
                nc.tensor.matmul(
                    S_t[:, h * 512:h * 512 + w],
                    KT[b][h * DH:(h + 1) * DH, kb * P:(kb + 1) * P],
                    QT[b][h * DH:(h + 1) * DH, qs:qs + w],
                    start=True, stop=True)
            return (b, qc, kb, off, w, S_t)

        def emit_exp(b, qc, kb, off, w, S_t):
            pt = ptp.tile([P, 1024], f16, tag="pt", name=f"pt_{b}_{qc}_{kb}")
            col = b * NB + kb
            sv = S_t.rearrange("p (h w) -> p h w", h=2)[:, :, 0:w]
            pv = pt.rearrange("p (h w) -> p h w", h=2)[:, :, 0:w]
            nc.scalar.activation(pv, sv, AF.Exp,
                                 bias=madd_sb[:, col:col + 1], scale=SCALE)
            if kb >= 4 * qc:  # chunk starts at the diagonal block
                cv = pt.rearrange("p (h w) -> p h w", h=2)[:, :, 0:P]
                nc.vector.tensor_tensor(
                    cv, cv, caus01[:, None, :].to_broadcast([P, 2, P]),
                    ALU.mult)
            return (b, qc, kb, off, w, pt)

        O_tiles = {}
        PL_tiles = {}
        norm_pending = []        # (b, qc, [(o_sb, zrow) per h]) to normalize

        def emit_O(b, qc, kb, off, w, pt):
            for h in range(HPC):
                if kb == 0:
                    O_tiles[(b, h, qc)] = psO.tile(
                        [DH + 1, 512], f32, tag="o", name=f"O_{b}_{h}_{qc}")
                O_ps = O_tiles[(b, h, qc)]
                nc.tensor.matmul(
                    O_ps[:, off:512],
                    VF[b][:, kb, h * (DH + 1):(h + 1) * (DH + 1)],
                    pt[:, h * 512:h * 512 + w],
                    start=(kb == 0), stop=(kb == 4 * qc + 3),
                )
            if kb == 4 * qc + 3:
                # Evacuate O + Z rows promptly (frees the PSUM banks for the
                # next qc); the rest of the normalize chain is deferred via
                # norm_pending so the PE's O stream never queues behind it.
                ev = []
                for h in range(HPC):
                    O_ps = O_tiles.pop((b, h, qc))
                    o_sb = zp.tile([DH, 512], f16, tag="osb", bufs=4,
                                   name=f"osb_{b}_{h}_{qc}")
                    nc.vector.tensor_copy(o_sb, O_ps[0:DH, :])
                    zrow = zp.tile([1, 512], f32, tag="zrow", bufs=2,
                                   name=f"zrow_{b}_{h}_{qc}")
                    nc.vector.tensor_copy(zrow, O_ps[DH:DH + 1, :])
                    ev.append((o_sb, zrow))
                norm_pending.append((b, qc, ev))

        # Deferred softmax normalization for one completed qc.
        # reciprocal_approx_fast (~51 ULP) instead of the bit-exact
        # iterative divide (~6 cycles/elem = 3.3us per row).
        def emit_norm(b, qc, ev):
            PLq = plp.tile([P, 512], f16, tag="PL", bufs=2 * NCH,
                           name=f"PL_{b}_{qc}")
            PL_tiles[(b, qc)] = PLq
            for h, (o_sb, zrow) in enumerate(ev):
                zinv = zp.tile([1, 512], f32, tag="zinv", bufs=2,
                               name=f"zinv_{b}_{h}_{qc}")
                nc.vector.reciprocal_approx_fast(zinv, zrow)
                rb = zp.tile([DH, 512], f32, tag="rb", bufs=2,
                             name=f"rb_{b}_{h}_{qc}")
                nc.gpsimd.partition_broadcast(rb, zinv)
                nc.vector.tensor_tensor(
                    PLq[h * DH:(h + 1) * DH, :], o_sb, rb, ALU.mult)

        # Output projection for one 128-token block.  Both 512-halves land
        # in one 2-bank "s" slot (single-instruction evacuation); successive
        # blocks alternate "s"/"mm"-pair for a deeper effective rotation.
        def emit_proj_tb(b, qc, tb, evac_eng):
            PLq = PL_tiles[(b, qc)]
            t = qc * 4 + tb
            y_sb = ysb.tile([P, D], f16, tag="ysb", name=f"y_{b}_{t}")
            y_ps = psS.tile([P, 1024], f32, tag="s", name=f"yps_{b}_{t}")
            for dc in range(2):
                nc.tensor.matmul(y_ps[:, dc * 512:(dc + 1) * 512],
                                 PLq[:, tb * P:(tb + 1) * P],
                                 wo_sb[:, dc * 512:(dc + 1) * 512],
                                 start=True, stop=True)
            ecopy(evac_eng, y_sb, y_ps)
            r0 = b * N + t * P
            nc.sync.dma_start(y[r0:r0 + P, :], y_sb)
            if tb == 3:
                del PL_tiles[(b, qc)]

        # ---------------- emission schedule ----------------------------
        # Phase 1: QKV + rotary + transposes for batch 0 only, one chunk at
        # a time; rotary+transposes lag 2 chunks behind the QKV matmuls so
        # the PE never waits on the cross-engine rotary chain.
        emit_vf_ones(0)
        emit_vf_ones(1)
        rotq = []                # (c, qsb_c) awaiting rotary+transpose
        for c in range(NCH):
            qsb_c = emit_qkv_chunk_mm(0, c)
            rotq.append((c, qsb_c))
            if len(rotq) > 2:
                cc, qq = rotq.pop(0)
                qkb = emit_rotary_chunk(0, cc, qq)
                emit_tr_chunk(0, cc, qkb, (nc.scalar, nc.vector))
        while rotq:
            cc, qq = rotq.pop(0)
            qkb = emit_rotary_chunk(0, cc, qq)
            emit_tr_chunk(0, cc, qkb, (nc.scalar, nc.vector))

        # Batch-1 QKV work items, woven one per pair into batch-0 attention
        # ("mm" PSUM, DVE evac/copies, cos-multiply on Pool -- ScalarE is
        # saturated by exp there).
        def b1_qkv_items():
            qsb_cs = {}
            qkbs = {}
            for c in range(NCH):
                for tb in range(4):
                    def mm(c=c, tb=tb):
                        if tb == 0:
                            qsb_cs[c] = qsb.tile([P, 4, F3], f16, tag="qs",
                                                 name=f"qsb_1_{c}")
                        emit_qkv_block_mm(1, c, tb, qsb_cs[c], "mm", nc.vector)
                    yield mm
                def rot(c=c):
                    qkbs[c] = emit_rotary_chunk(1, c, qsb_cs[c],
                                                cos_eng=nc.gpsimd)
                yield rot
                for tb in range(4):
                    def tr(c=c, tb=tb):
                        emit_tr_block(1, c, tb, qkbs[c], (nc.vector,))
                    yield tr

        # Attention for one batch; optional per-pair weave items; optional
        # projection drain (1 per 2 pairs).
        O_LAG = 3

        def run_attention(b, weave=None, proj_drain=None):
            pend = []
            normq = []
            for i, (qc, kb) in enumerate(attn_pairs(b)):
                su = emit_S(b, qc, kb)
                pend.append(emit_exp(*su))
                if weave is not None:
                    nxt = next(weave, None)
                    if nxt is not None:
                        nxt()
                if len(pend) > O_LAG:
                    if normq:
                        emit_norm(*normq.pop(0))
                    emit_O(*pend.pop(0))
                    normq.extend(norm_pending)
                    norm_pending.clear()
                if proj_drain is not None and i % 2 == 1:
                    nxt = next(proj_drain, None)
                    if nxt is not None:
                        nxt()
            while pend:
                emit_O(*pend.pop(0))
            normq.extend(norm_pending)
            norm_pending.clear()
            while normq:
                emit_norm(*normq.pop(0))

        def proj_items(b, engs):
            for i, (qc, tb) in enumerate(
                    [(qc, tb) for qc in range(NCH) for tb in range(4)]):
                def pj(qc=qc, tb=tb, i=i):
                    emit_proj_tb(b, qc, tb, engs[i % len(engs)])
                yield pj

        # Phase 2: batch-0 attention with batch-1 QKV woven in.
        run_attention(0, weave=b1_qkv_items())
        # Phase 3: batch-1 attention with batch-0 projections woven in.
        run_attention(1, proj_drain=proj_items(0, (nc.vector,)))
        # Phase 4: batch-1 projections.
        for pj in proj_items(1, (nc.scalar, nc.vector)):
            pj()

    nc.compile()
    return nc


_PERM = np.concatenate([np.arange(0, DH, 2), np.arange(1, DH, 2)])  # deint


def prep_inputs(x, mask, rotary_pos_emb, W_qkv, W_out, dt_mode="f16"):
    """Host-side shard prep: per-core input dicts (layout/permutation only,
    plus mask encode and rotary cos/sin table build)."""
    x = np.asarray(x, dtype=np.float32)
    W_qkv = np.asarray(W_qkv, dtype=np.float32)
    W_out = np.asarray(W_out, dtype=np.float32)
    rope = np.asarray(rotary_pos_emb, dtype=np.float32)
    mask = np.asarray(mask)

    xT = np.ascontiguousarray(x.reshape(NT, D).T.astype(np.float16))
    madd = np.where(mask, np.float32(0.0), np.float32(NEG)).astype(np.float32)
    madd_dev = np.ascontiguousarray(
        madd.reshape(B, NB, P).transpose(2, 0, 1).reshape(P, B * NB))
    kidx = np.arange(P)[:, None]
    qidx = np.arange(P)[None, :]
    caus = (qidx >= kidx).astype(np.float16)          # 0/1 multiplicative
    freq = rope[-N:, :]                               # [N, 64]
    cosd = np.cos(freq)[:, _PERM].astype(np.float16)
    # sind partner layout: slot [0:32] = -sin(even freqs), [32:64] = +sin(odd)
    sind = np.concatenate([-np.sin(freq[:, 0::2]), np.sin(freq[:, 1::2])],
                          axis=1).astype(np.float16)
    cosd = np.ascontiguousarray(cosd)
    sind = np.ascontiguousarray(sind)

    in_maps = []
    for c in range(NCORES):
        rows = []
        for tsel in range(3):                      # q, k, v row blocks
            for h in (HPC * c, HPC * c + 1):
                o = tsel * H * DH + h * DH
                rows.append(W_qkv[o + _PERM, :])
        wqkvT = np.ascontiguousarray(
            np.concatenate(rows, axis=0).T.astype(np.float16))
        cols = np.concatenate([FD * c + h * DH + _PERM for h in range(HPC)])
        woT = np.ascontiguousarray(W_out[:, cols].T.astype(np.float16))
        in_maps.append({
            "xT": xT, "wqkvT": wqkvT, "woT": woT,
            "cosd": cosd, "sind": sind, "madd": madd_dev, "caus": caus,
        })
    return in_maps


def _ensure_ntff_hook():
    """Install antenv.axon_hooks + the ctypes NTFF profile hook if the image
    lacks them (needed only for trace=True timing runs, not for kernel())."""
    import types
    try:
        from antenv.axon_hooks import get_axon_ntff_profile_hook  # noqa: F401
        return
    except ImportError:
        pass
    try:
        import antenv
        mod = types.ModuleType("antenv.axon_hooks")
        _state = {"hook": None}

        def set_axon_ntff_profile_hook(h):
            _state["hook"] = h

        def get_axon_ntff_profile_hook():
            return _state["hook"]

        mod.set_axon_ntff_profile_hook = set_axon_ntff_profile_hook
        mod.get_axon_ntff_profile_hook = get_axon_ntff_profile_hook
        sys.modules["antenv.axon_hooks"] = mod
        antenv.axon_hooks = mod
        from trn_agent_boot.trn_boot import _ntff_profile_via_ctypes
        hook = _ntff_profile_via_ctypes("/opt/axon/libaxon_pjrt.so")
        if hook is not None:
            set_axon_ntff_profile_hook(hook)
    except Exception as e:  # degrade to untimed runs
        print(f"ntff hook install failed: {e!r}", file=sys.stderr)


_NC_CACHE = {}


def _get_nc(dt_mode="f16"):
    if "nc" not in _NC_CACHE:
        _NC_CACHE["nc"] = build_nc()
    return _NC_CACHE["nc"]


def run_cores(in_maps, trace=False, dt_mode="f16"):
    if trace:
        _ensure_ntff_hook()
    nc = _get_nc(dt_mode)
    res = run_bass_kernel_spmd(
        nc, in_maps, core_ids=list(range(NCORES)), trace=trace,
        trace_cores=list(range(NCORES)) if trace else None,
    )
    return res


DT_MODE = "f16"


def kernel(x, mask, rotary_pos_emb, W_qkv, W_out, b_out):
    in_maps = prep_inputs(x, mask, rotary_pos_emb, W_qkv, W_out)
    res = run_cores(in_maps, trace=False)
    y = np.zeros((NT, D), dtype=np.float32)
    for r in res.results:
        y += r["y"]
    y += np.asarray(b_out, dtype=np.float32)[None, :]
    return y.reshape(B, N, D)
